# revision 54
# baseline (speedup 1.0000x reference)
"""LinearRNN final-state kernel for 8 Trainium2 NeuronCores.

Reference computation:
    u_t = Wxh @ x_t + bxh            (input projection)
    h_t = u_t + Whh @ h_{t-1}        (recurrence over T=1024 steps)
    return h_T                        -> [B=32, H=512]

The recurrence is linear:  h_T = sum_t u_t @ A^(T-1-t),  A = Whh^T
(row-vector convention).  Structure (driven by the ~58ns/instruction PE
floor: matmuls below ~139 moving columns are instruction-bound):

  * radix-4 fused projection: 4 consecutive timesteps fold directly into
    the input GEMM via W, WA, WA^2, WA^3 (W = Wxh^T), absorbing the first
    two tree levels; the bias enters here as b(I+A+A^2+A^3).
  * wide per-half tree levels with A^4..A^64 collapse each half of the
    sequence to 4 blocks of 128 timesteps per batch row (matmuls stay
    >=64 wide);
  * a Horner chain over the 8 blocks with stationary A^128:
    h <- h @ A^128 + V_k, interleaved with the tail of the tree;
  * only A^2..A^128 are materialized (7 squaring products).  The product
    chain is the latency backbone: projections and tree levels are
    threaded between its links, and each product's lhsT-layout transpose
    rides the DMA xbar engine (14ns per 16x128 tile) per-mcc-row so it
    overlaps the PSUM copies.  x is converted f32->bf16 on the GPSIMD
    engine and transposed by the DMA xbar, so the PE does no transpose
    or conversion work for x at all.

Everything on the PE runs in bf16 (1 cycle/row at any width; empirically
rel-err ~7e-3 vs the 2e-2 budget).

Sharding: data-parallel over batch (B=32 -> 4 rows/core on 8 cores);
weights and the squaring chain are replicated.
"""

import numpy as np

B, T, IN, H = 32, 1024, 256, 512
NCORES = 8
BC = B // NCORES          # 4 batch rows per core
COLS = BC * T             # 4096 sequence columns per core
HC = H // 128             # 4 hidden-dim chunks of 128
ICH = IN // 128           # 2 input-dim chunks
NG = COLS // 512          # 8 x-groups (each = half of one batch row)

_cache: dict = {}


def _build():
    import concourse.bass as bass
    import concourse.mybir as mybir
    from concourse import bacc
    from concourse.tile import TileContext
    from concourse.masks import make_identity

    f32 = mybir.dt.float32
    bf16 = mybir.dt.bfloat16
    ACT_COPY = mybir.ActivationFunctionType.Copy
    ACT_IDENT = mybir.ActivationFunctionType.Identity

    nc = bacc.Bacc(None)
    x_d = nc.declare_dram_parameter("x", [COLS, IN], f32, isOutput=False)
    wxh_d = nc.declare_dram_parameter("Wxh", [H, IN], f32, isOutput=False)
    bxh_d = nc.declare_dram_parameter("bxh", [H], f32, isOutput=False)
    whh_d = nc.declare_dram_parameter("Whh", [H, H], f32, isOutput=False)
    # Output stays in on-chip layout [128, HC*BC]; host unscrambles.
    out_d = nc.declare_dram_parameter("h_out", [128, HC * BC], f32, isOutput=True)

    with TileContext(nc) as tc:
        with (
            tc.tile_pool(name="const", bufs=1) as cpool,
            tc.tile_pool(name="xload", bufs=4) as xpool,
            tc.tile_pool(name="vbuf", bufs=1) as vpool,
            tc.tile_pool(name="mm", bufs=2, space="PSUM") as mmpool,
            tc.tile_pool(name="tr", bufs=2, space="PSUM") as trpool,
            tc.tile_pool(name="pj", bufs=2, space="PSUM") as pjpool,
            tc.tile_pool(name="tl", bufs=2, space="PSUM") as tlpool,
        ):
            ident_b = cpool.tile([128, 128], bf16, tag="identb")
            make_identity(nc, ident_b[:])

            # PE warm-up: keeps the PE busy through the initial weight-DMA
            # wait and completes the clock ramp before real work arrives.
            warm = mmpool.tile([128, H], f32, tag="mm")
            for _ in range(48):
                nc.tensor.matmul(
                    warm[:, 0:128], ident_b[:], ident_b[:], start=True, stop=True
                )

            # ---- loads -------------------------------------------------
            # Whh and Wxh ride the SP ring ahead of the x loads so they
            # win the DMA device first: the whole setup chain hangs off
            # them.  Whh comes in two chunks so conversion overlaps the
            # transfer.
            w_f32 = cpool.tile([128, HC, H], f32, tag="wf32")
            whh_r = whh_d.rearrange("(c p) f -> p c f", p=128)
            nc.sync.dma_start(w_f32[:, 0:2, :], whh_r[:, 0:2, :])
            nc.sync.dma_start(w_f32[:, 2:4, :], whh_r[:, 2:4, :])
            wxh_f32 = cpool.tile([128, HC, IN], f32, tag="wxf32")
            nc.sync.dma_start(
                wxh_f32[:], wxh_d.rearrange("(c p) f -> p c f", p=128)
            )
            b_f32 = cpool.tile([128, HC], f32, tag="bf32")
            nc.scalar.dma_start(b_f32[:], bxh_d.rearrange("(c p) -> p c", p=128))

            # x groups; group g = half (g%2) of batch row g//2.  The first
            # half of every row is needed first.
            load_order = [0, 2, 4, 6, 1, 3, 5, 7]
            xg_f32 = {}
            for g in load_order:
                xg = xpool.tile([128, 4, IN], f32, tag="xg")
                nc.sync.dma_start(
                    xg[:],
                    x_d[g * 512:(g + 1) * 512, :].rearrange(
                        "(j p) i -> p j i", p=128
                    ),
                )
                xg_f32[g] = xg

            # ---- bf16 conversions of weights --------------------------
            w_bf = cpool.tile([128, HC, H], bf16, tag="wbf")  # Whh = A^T natural
            for c in range(HC):
                nc.vector.tensor_copy(w_bf[:, c, :], w_f32[:, c, :])
            wxh_bf = cpool.tile([128, HC, IN], bf16, tag="wxbf")
            nc.scalar.activation(wxh_bf[:], wxh_f32[:], ACT_COPY)
            # all x conversions up front on the Act ring: their DMAs all
            # land by ~12us and everything downstream queues behind them
            # on this in-order engine.
            b_bf = cpool.tile([128, HC, 1], bf16, tag="bbf")
            nc.scalar.activation(b_bf[:, :, 0], b_f32[:], ACT_COPY)

            # ---- x path: convert to bf16 (Act), transpose via DMA xbar -
            # xT_all[q, g, gb, p] = x_bf[512 g + (gb//2)*128 + p,
            #                            (gb%2)*128 + q]
            xT_all = cpool.tile([128, NG, 8, 128], bf16, tag="xT")
            xg_bf = {}

            def emit_conv(g):
                # f32->bf16 on the otherwise-idle GPSIMD engine (SBUF to
                # SBUF, which Pool is allowed to touch)
                xb = xpool.tile([128, 4, IN], bf16, tag="xgbf")
                nc.gpsimd.tensor_copy(xb[:], xg_f32[g][:])
                xg_bf[g] = xb

            def emit_xpose(g):
                nc.sync.dma_start_transpose(xT_all[:, g, :, :], xg_bf[g][:])

            for g in (0, 2, 4, 6, 1, 3, 5, 7):
                emit_conv(g)
            G1T = cpool.tile([128, 8, 128], bf16, tag="G1T")
            for g in (0, 2, 4, 6):
                emit_xpose(g)
            # G1T rides the SP ring here: it must not queue behind the
            # half-1 xposes (whose conversions land later than G1)
            def emit_g1t(G1):
                nc.sync.dma_start_transpose(G1T[:], G1[:])

            def transpose_quad(dst_ap, srcs):
                """PE transpose of up to four [128,128] bf16 blocks through
                one PSUM tile + a single wide DVE copy."""
                tp = trpool.tile([128, 4, 128], bf16, tag="tp")
                for i, s in enumerate(srcs):
                    nc.tensor.transpose(tp[:, i, :], s, ident_b[:])
                nc.vector.tensor_copy(dst_ap, tp[:, :len(srcs), :])

            # S1 = A natural: S1[p, kc, f] = A[kc*128+p, f] = Whh[f, kc*128+p]
            S1 = cpool.tile([128, HC, H], bf16, tag="S1")
            for kc in range(HC):
                transpose_quad(
                    S1[:, kc, :],
                    [w_bf[:, rc, kc * 128:(kc + 1) * 128] for rc in range(HC)],
                )


            # lhsT accessors: natural [128, HC, H] tiles slice
            # [:, kc, mcc-block]; DMA-transposed tiles are [128, 16, 128]
            # and slice [:, 4*mcc + kc, :].
            def nat(tile):
                return lambda kc, mcc: tile[:, kc, mcc * 128:(mcc + 1) * 128]

            def xbar(tile):
                return lambda kc, mcc: tile[:, 4 * mcc + kc, :]

            def square(U_sl, S, name, copy_eng="dve", U_out=None):
                """A^{2m}: out[:, mcc, :] = sum_kc U_sl(kc,mcc)^T @ S[:,kc,:].
                With U_out, each mcc row is DMA-xbar-transposed into the
                next link's lhsT tile as soon as its copy lands."""
                Snew = cpool.tile([128, HC, H], bf16, tag=name)
                for mcc in range(HC):
                    ps = mmpool.tile([128, H], f32, tag="mm")
                    for kc in range(HC):
                        nc.tensor.matmul(
                            ps[:],
                            U_sl(kc, mcc),
                            S[:, kc, :],
                            start=(kc == 0),
                            stop=(kc == HC - 1),
                        )
                    if copy_eng == "act" or (copy_eng == "both" and mcc % 2):
                        nc.scalar.activation(Snew[:, mcc, :], ps[:], ACT_COPY)
                    else:
                        nc.vector.tensor_copy(Snew[:, mcc, :], ps[:])
                    if U_out is not None:
                        nc.sync.dma_start_transpose(
                            U_out[:, 4 * mcc:4 * mcc + 4, :],
                            Snew[:, mcc, :],
                        )
                return Snew

            def pe_transpose_mat(S, name):
                U = cpool.tile([128, HC, H], bf16, tag=name)
                for kc in range(HC):
                    transpose_quad(
                        U[:, kc, :],
                        [S[:, fc, kc * 128:(kc + 1) * 128] for fc in range(HC)],
                    )
                return nat(U)

            def c_round(c_prev, S_m, name):
                """c_{2m} = c_m + c_m @ A^m  (column form)."""
                psf = tlpool.tile([128, HC, 128], f32, tag="tl", name="csm")
                ps = psf[:, :, 0:1]
                for mcc in range(HC):
                    for kc in range(HC):
                        nc.tensor.matmul(
                            ps[:, mcc, :],
                            S_m[:, kc, mcc * 128:(mcc + 1) * 128],
                            c_prev[:, kc, :],
                            start=(kc == 0),
                            stop=(kc == HC - 1),
                        )
                c_new = cpool.tile([128, HC, 1], bf16, tag=name)
                nc.vector.tensor_add(c_new[:], ps[:], c_prev[:])
                return c_new

            def g_mats(lhs_sl, rhs, name, copy_eng="act"):
                """G[:, oc, :] = sum_kc lhs_sl(kc,oc)^T @ rhs(kc), 512 wide."""
                G = cpool.tile([128, ICH, H], bf16, tag=name)
                for oc in range(ICH):
                    ps = mmpool.tile([128, H], f32, tag="mm")
                    for kc in range(HC):
                        nc.tensor.matmul(
                            ps[:],
                            lhs_sl(kc, oc),
                            rhs(kc),
                            start=(kc == 0),
                            stop=(kc == HC - 1),
                        )
                    if copy_eng == "act":
                        nc.scalar.activation(G[:, oc, :], ps[:], ACT_COPY)
                    else:
                        nc.vector.tensor_copy(G[:, oc, :], ps[:])
                return G

            # ---- squaring chain / G matrices / bias vector -------------
            # The product chain A^2..A^128 is the latency backbone; all
            # projection and tree work is threaded between its links so
            # the PSUM->SBUF copies and DMA xbar transposes cost no PE
            # time.  Only the 7 Horner rounds remain as a serial tail.
            S2 = square(nat(w_bf), S1, "S2")        # U1 = Whh natural
            # Wq = Wxh^T as lhsT (fills the PE while S2's copies land)
            Wq = cpool.tile([128, ICH, H], bf16, tag="Wq")
            for ic in range(ICH):
                transpose_quad(
                    Wq[:, ic, :],
                    [wxh_bf[:, rc, ic * 128:(ic + 1) * 128] for rc in range(HC)],
                )
            # G1 = W A  (lhsT chunks = Wxh natural rows of wxh_bf)
            G1 = g_mats(
                lambda kc, oc: wxh_bf[:, kc, oc * 128:(oc + 1) * 128],
                lambda kc: S1[:, kc, :], "G1",
            )
            emit_g1t(G1)
            c2 = c_round(b_bf, S1, "c2")
            U2 = pe_transpose_mat(S2, "U2")
            G2 = g_mats(
                lambda kc, oc: wxh_bf[:, kc, oc * 128:(oc + 1) * 128],
                lambda kc: S2[:, kc, :], "G2", copy_eng="dve",
            )
            S4 = square(U2, S2, "S4")
            # G3 = G1 A^2 = W A^3
            G3 = g_mats(
                lambda kc, oc: G1T[:, 4 * oc + kc, :],
                lambda kc: S2[:, kc, :], "G3", copy_eng="dve",
            )
            U4 = pe_transpose_mat(S4, "U4")
            c4 = c_round(c2, S2, "c4")              # = b(I+A+A^2+A^3)

            PROJ_MATS = [Wq, G1, G2, G3]  # applied to t ≡ 3,2,1,0 (mod 4)
            # ---- main pipeline -----------------------------------------
            # Per x-group (512 timesteps of one batch row): radix-4 fused
            # projection -> 128 quad-columns.  Per half (4 groups, one per
            # row): tree levels A^4..A^64 collapse 512 -> 16 columns
            # (4 blocks of 128 timesteps x 4 rows, b-major).  A Horner
            # chain joins the 8 blocks with A^128.
            vhalf = {}
            v5 = {}

            pj_ps = {}

            def emit_proj(g, mccs=(0, 1, 2, 3)):
                half, row = g % 2, g // 2
                if half not in vhalf:
                    vhalf[half] = vpool.tile(
                        [128, HC, 512], bf16, tag=f"v0h{half}",
                        name=f"v0h{half}",
                    )
                if g in pj_ps:
                    pp = pj_ps.pop(g)
                else:
                    pp = pjpool.tile([128, HC, 128], f32, tag="pj")
                    pj_ps[g] = pp
                for mcc in mccs:
                    first = True
                    for m in range(4):
                        for ic in range(ICH):
                            nc.tensor.matmul(
                                pp[:, mcc, :],
                                PROJ_MATS[m][:, ic, mcc * 128:(mcc + 1) * 128],
                                xT_all[:, g, ic::2, 3 - m::4],
                                start=first,
                                stop=(m == 3 and ic == ICH - 1),
                            )
                            first = False
                # epilogue adds the radix-4 bias b(I+A+A^2+A^3)
                for mcc in mccs:
                    nc.scalar.activation(
                        vhalf[half][:, mcc, row * 128:(row + 1) * 128],
                        pp[:, mcc, :],
                        ACT_IDENT,
                        bias=c4[:, mcc, :],
                    )

            def tree_level(src, n_in, S_m, name, sub=None, dst=None):
                """One binary level over b-major cols: src [128, HC, n_in]
                -> dst [128, HC, n_in//2].  sub=(lo,hi) restricts batch
                rows (to keep a 512-col level's PSUM in one bank)."""
                b_lo, b_hi = sub if sub else (0, BC)
                nb = b_hi - b_lo
                per = n_in // BC
                srcr = src[:].rearrange("p c (b j) -> p c b j", b=BC)
                ps = tlpool.tile([128, HC, 128], f32, tag="tl")
                w = nb * per // 2
                for mcc in range(HC):
                    for kc in range(HC):
                        nc.tensor.matmul(
                            ps[:, mcc, 0:w],
                            S_m[:, kc, mcc * 128:(mcc + 1) * 128],
                            srcr[:, kc, b_lo:b_hi, 0::2],
                            start=(kc == 0),
                            stop=(kc == HC - 1),
                        )
                if dst is None:
                    dst = vpool.tile(
                        [128, HC, n_in // 2], bf16, tag=name, name=name
                    )
                dstr = dst[:].rearrange("p c (b j) -> p c b j", b=BC)
                nc.vector.tensor_add(
                    dstr[:, :, b_lo:b_hi, :],
                    ps[:, :, 0:w].rearrange("p c (b j) -> p c b j", b=nb),
                    srcr[:, :, b_lo:b_hi, 1::2],
                )
                return dst

            hs = {}

            def emit_horner(r):
                """h_r = h_{r-1} @ A^128 + V_r (V_r = row-block r%4 of
                half r//4); round 7 writes the f32 output tile."""
                vsrc = v5[r // 4][:].rearrange("p c (b j) -> p c b j", b=BC)
                rhs = (
                    v5[0][:].rearrange("p c (b j) -> p c b j", b=BC)
                    if r == 1 else hs[r - 1][:]
                )
                psf = tlpool.tile(
                    [128, HC, 128], f32, tag="tl", name=f"hsm{r % 2}"
                )
                ps = psf[:, :, 0:4]
                for mcc in range(HC):
                    for kc in range(HC):
                        nc.tensor.matmul(
                            ps[:, mcc, :],
                            S128[:, kc, mcc * 128:(mcc + 1) * 128],
                            rhs[:, kc, :, 0] if r == 1 else rhs[:, kc, :],
                            start=(kc == 0),
                            stop=(kc == HC - 1),
                        )
                if r < T // 128 - 1:
                    h = vpool.tile(
                        [128, HC, BC], bf16, tag="h", bufs=3, name="h"
                    )
                    nc.vector.tensor_add(h[:], ps[:], vsrc[:, :, :, r % 4])
                    hs[r] = h
                else:
                    # final round: split the add and the store so the
                    # first DMA's descriptor-generation pipeline overlaps
                    # the second half's add
                    fout = cpool.tile([128, HC, BC], f32, tag="fout")
                    o_r = out_d.rearrange("p (c b) -> p c b", b=BC)
                    nc.vector.tensor_add(
                        fout[:, 0:2, :], ps[:, 0:2, :], vsrc[:, 0:2, :, r % 4]
                    )
                    nc.scalar.dma_start(o_r[:, 0:2, :], fout[:, 0:2, :])
                    nc.vector.tensor_add(
                        fout[:, 2:4, :], ps[:, 2:4, :], vsrc[:, 2:4, :, r % 4]
                    )
                    nc.sync.dma_start(o_r[:, 2:4, :], fout[:, 2:4, :])
                    hs[r] = fout

            # thread everything through the product chain: ~one
            # projection pair fills each U transpose window; the tree
            # levels run as two alternating half-ladders so their
            # PSUM round trips hide behind each other
            U8t = cpool.tile([128, 16, 128], bf16, tag="U8")
            U16t = cpool.tile([128, 16, 128], bf16, tag="U16")
            U32t = cpool.tile([128, 16, 128], bf16, tag="U32")
            U64t = cpool.tile([128, 16, 128], bf16, tag="U64")
            emit_proj(0)
            S8 = square(U4, S4, "S8", U_out=U8t)
            U8 = xbar(U8t)
            # half-1 x transposes ride SP here: after U8's per-mcc
            # transposes (which gate S16) but well before proj(1) needs
            # them
            for g in (1, 3, 5, 7):
                emit_xpose(g)
            emit_proj(2)
            emit_proj(4)
            S16 = square(U8, S8, "S16", copy_eng="both", U_out=U16t)
            U16 = xbar(U16t)
            emit_proj(6)
            emit_proj(1)
            S32 = square(U16, S16, "S32", copy_eng="both", U_out=U32t)
            U32 = xbar(U32t)
            emit_proj(3)
            emit_proj(5)
            S64 = square(U32, S32, "S64", copy_eng="both", U_out=U64t)
            U64 = xbar(U64t)
            emit_proj(7)
            v0h0, v0h1 = vhalf[0], vhalf[1]
            v1h0 = tree_level(v0h0, 512, S4, "v1h0", sub=(0, 2))
            tree_level(v0h0, 512, S4, "v1h0", sub=(2, 4), dst=v1h0)
            S128 = square(U64, S64, "S128", copy_eng="both")
            v1h1 = tree_level(v0h1, 512, S4, "v1h1", sub=(0, 2))
            tree_level(v0h1, 512, S4, "v1h1", sub=(2, 4), dst=v1h1)
            v2h0 = tree_level(v1h0, 256, S8, "v2h0")
            v2h1 = tree_level(v1h1, 256, S8, "v2h1")
            v3h0 = tree_level(v2h0, 128, S16, "v3h0")
            v3h1 = tree_level(v2h1, 128, S16, "v3h1")
            v4h0 = tree_level(v3h0, 64, S32, "v4h0")
            v4h1 = tree_level(v3h1, 64, S32, "v4h1")
            v5[0] = tree_level(v4h0, 32, S64, "v5h0")
            emit_horner(1)
            v5[1] = tree_level(v4h1, 32, S64, "v5h1")
            emit_horner(2)
            for r in range(3, 8):
                emit_horner(r)



    nc.compile()
    return nc


def _get_nc():
    if "nc" not in _cache:
        _cache["nc"] = _build()
    return _cache["nc"]


def _in_maps(inputs):
    x = np.ascontiguousarray(np.asarray(inputs["x"], dtype=np.float32))
    wxh = np.ascontiguousarray(np.asarray(inputs["Wxh"], dtype=np.float32))
    bxh = np.ascontiguousarray(np.asarray(inputs["bxh"], dtype=np.float32))
    whh = np.ascontiguousarray(np.asarray(inputs["Whh"], dtype=np.float32))
    return [
        dict(
            x=np.ascontiguousarray(
                x[c * BC:(c + 1) * BC].reshape(COLS, IN)
            ),
            Wxh=wxh,
            bxh=bxh,
            Whh=whh,
        )
        for c in range(NCORES)
    ]


def kernel(**inputs) -> np.ndarray:
    from concourse.bass_utils import run_bass_kernel_spmd

    res = run_bass_kernel_spmd(
        _get_nc(), _in_maps(inputs), list(range(NCORES))
    ).results
    return _assemble(res)


def _assemble(results) -> np.ndarray:
    outs = []
    for c in range(NCORES):
        o = np.asarray(results[c]["h_out"])      # [128, HC*BC] on-chip layout
        o = o.reshape(128, HC, BC).transpose(2, 1, 0).reshape(BC, H)
        outs.append(o)
    return np.concatenate(outs, axis=0).astype(np.float32)


# revision 55
# speedup vs baseline: 1.0078x; 1.0078x over previous
"""LinearRNN final-state kernel for 8 Trainium2 NeuronCores.

Reference computation:
    u_t = Wxh @ x_t + bxh            (input projection)
    h_t = u_t + Whh @ h_{t-1}        (recurrence over T=1024 steps)
    return h_T                        -> [B=32, H=512]

The recurrence is linear:  h_T = sum_t u_t @ A^(T-1-t),  A = Whh^T
(row-vector convention).  Structure (driven by the ~58ns/instruction PE
floor: matmuls below ~139 moving columns are instruction-bound):

  * radix-4 fused projection: 4 consecutive timesteps fold directly into
    the input GEMM via W, WA, WA^2, WA^3 (W = Wxh^T), absorbing the first
    two tree levels; the bias enters here as b(I+A+A^2+A^3).
  * wide per-half tree levels with A^4..A^64 collapse each half of the
    sequence to 4 blocks of 128 timesteps per batch row (matmuls stay
    >=64 wide);
  * a Horner chain over the 8 blocks with stationary A^128:
    h <- h @ A^128 + V_k, interleaved with the tail of the tree;
  * only A^2..A^128 are materialized (7 squaring products).  The product
    chain is the latency backbone: projections and tree levels are
    threaded between its links, and each product's lhsT-layout transpose
    rides the DMA xbar engine (14ns per 16x128 tile) per-mcc-row so it
    overlaps the PSUM copies.  x is converted f32->bf16 on the GPSIMD
    engine and transposed by the DMA xbar, so the PE does no transpose
    or conversion work for x at all.

Everything on the PE runs in bf16 (1 cycle/row at any width; empirically
rel-err ~7e-3 vs the 2e-2 budget).

Sharding: data-parallel over batch (B=32 -> 4 rows/core on 8 cores);
weights and the squaring chain are replicated.
"""

import numpy as np

B, T, IN, H = 32, 1024, 256, 512
NCORES = 8
BC = B // NCORES          # 4 batch rows per core
COLS = BC * T             # 4096 sequence columns per core
HC = H // 128             # 4 hidden-dim chunks of 128
ICH = IN // 128           # 2 input-dim chunks
NG = COLS // 512          # 8 x-groups (each = half of one batch row)

_cache: dict = {}


def _build():
    import concourse.bass as bass
    import concourse.mybir as mybir
    from concourse import bacc
    from concourse.tile import TileContext
    from concourse.masks import make_identity

    f32 = mybir.dt.float32
    bf16 = mybir.dt.bfloat16
    ACT_COPY = mybir.ActivationFunctionType.Copy
    ACT_IDENT = mybir.ActivationFunctionType.Identity

    nc = bacc.Bacc(None)
    x_d = nc.declare_dram_parameter("x", [COLS, IN], f32, isOutput=False)
    wxh_d = nc.declare_dram_parameter("Wxh", [H, IN], f32, isOutput=False)
    bxh_d = nc.declare_dram_parameter("bxh", [H], f32, isOutput=False)
    whh_d = nc.declare_dram_parameter("Whh", [H, H], f32, isOutput=False)
    # Output stays in on-chip layout [128, HC*BC]; host unscrambles.
    out_d = nc.declare_dram_parameter("h_out", [128, HC * BC], f32, isOutput=True)

    with TileContext(nc) as tc:
        with (
            tc.tile_pool(name="const", bufs=1) as cpool,
            tc.tile_pool(name="xload", bufs=4) as xpool,
            tc.tile_pool(name="vbuf", bufs=1) as vpool,
            tc.tile_pool(name="mm", bufs=2, space="PSUM") as mmpool,
            tc.tile_pool(name="tr", bufs=2, space="PSUM") as trpool,
            tc.tile_pool(name="pj", bufs=2, space="PSUM") as pjpool,
            tc.tile_pool(name="tl", bufs=2, space="PSUM") as tlpool,
        ):
            ident_b = cpool.tile([128, 128], bf16, tag="identb")
            make_identity(nc, ident_b[:])

            # PE warm-up: keeps the PE busy through the initial weight-DMA
            # wait and completes the clock ramp before real work arrives.
            warm = mmpool.tile([128, H], f32, tag="mm")
            for _ in range(48):
                nc.tensor.matmul(
                    warm[:, 0:128], ident_b[:], ident_b[:], start=True, stop=True
                )

            # ---- loads -------------------------------------------------
            # Whh and Wxh ride the SP ring ahead of the x loads so they
            # win the DMA device first: the whole setup chain hangs off
            # them.  Whh comes in two chunks so conversion overlaps the
            # transfer.
            w_f32 = cpool.tile([128, HC, H], f32, tag="wf32")
            whh_r = whh_d.rearrange("(c p) f -> p c f", p=128)
            nc.sync.dma_start(w_f32[:, 0:2, :], whh_r[:, 0:2, :])
            nc.sync.dma_start(w_f32[:, 2:4, :], whh_r[:, 2:4, :])
            wxh_f32 = cpool.tile([128, HC, IN], f32, tag="wxf32")
            nc.sync.dma_start(
                wxh_f32[:], wxh_d.rearrange("(c p) f -> p c f", p=128)
            )
            b_f32 = cpool.tile([128, HC], f32, tag="bf32")
            nc.scalar.dma_start(b_f32[:], bxh_d.rearrange("(c p) -> p c", p=128))

            # x groups; group g = half (g%2) of batch row g//2.  The first
            # half of every row is needed first.
            load_order = [0, 2, 4, 6, 1, 3, 5, 7]
            xg_f32 = {}
            for g in load_order:
                xg = xpool.tile([128, 4, IN], f32, tag="xg")
                nc.sync.dma_start(
                    xg[:],
                    x_d[g * 512:(g + 1) * 512, :].rearrange(
                        "(j p) i -> p j i", p=128
                    ),
                )
                xg_f32[g] = xg

            # ---- bf16 conversions of weights --------------------------
            w_bf = cpool.tile([128, HC, H], bf16, tag="wbf")  # Whh = A^T natural
            for c in range(HC):
                nc.vector.tensor_copy(w_bf[:, c, :], w_f32[:, c, :])
            wxh_bf = cpool.tile([128, HC, IN], bf16, tag="wxbf")
            nc.scalar.activation(wxh_bf[:], wxh_f32[:], ACT_COPY)
            # all x conversions up front on the Act ring: their DMAs all
            # land by ~12us and everything downstream queues behind them
            # on this in-order engine.
            b_bf = cpool.tile([128, HC, 1], bf16, tag="bbf")
            nc.scalar.activation(b_bf[:, :, 0], b_f32[:], ACT_COPY)

            # ---- x path: convert to bf16 (Act), transpose via DMA xbar -
            # xT_all[q, g, gb, p] = x_bf[512 g + (gb//2)*128 + p,
            #                            (gb%2)*128 + q]
            xT_all = cpool.tile([128, NG, 8, 128], bf16, tag="xT")
            xg_bf = {}

            def emit_conv(g):
                # f32->bf16 on the otherwise-idle GPSIMD engine (SBUF to
                # SBUF, which Pool is allowed to touch)
                xb = xpool.tile([128, 4, IN], bf16, tag="xgbf")
                nc.gpsimd.tensor_copy(xb[:], xg_f32[g][:])
                xg_bf[g] = xb

            def emit_xpose(g):
                nc.sync.dma_start_transpose(xT_all[:, g, :, :], xg_bf[g][:])

            for g in (0, 2, 4, 6, 1, 3, 5, 7):
                emit_conv(g)
            G1T = cpool.tile([128, 8, 128], bf16, tag="G1T")
            for g in (0, 2, 4, 6):
                emit_xpose(g)
            # G1T rides the SP ring here: it must not queue behind the
            # half-1 xposes (whose conversions land later than G1)
            def emit_g1t(G1):
                nc.sync.dma_start_transpose(G1T[:], G1[:])

            def transpose_quad(dst_ap, srcs):
                """PE transpose of up to four [128,128] bf16 blocks through
                one PSUM tile + a single wide DVE copy."""
                tp = trpool.tile([128, 4, 128], bf16, tag="tp")
                for i, s in enumerate(srcs):
                    nc.tensor.transpose(tp[:, i, :], s, ident_b[:])
                nc.vector.tensor_copy(dst_ap, tp[:, :len(srcs), :])

            # S1 = A natural: S1[p, kc, f] = A[kc*128+p, f] = Whh[f, kc*128+p]
            S1 = cpool.tile([128, HC, H], bf16, tag="S1")
            for kc in range(HC):
                transpose_quad(
                    S1[:, kc, :],
                    [w_bf[:, rc, kc * 128:(kc + 1) * 128] for rc in range(HC)],
                )


            # lhsT accessors: natural [128, HC, H] tiles slice
            # [:, kc, mcc-block]; DMA-transposed tiles are [128, 16, 128]
            # and slice [:, 4*mcc + kc, :].
            def nat(tile):
                return lambda kc, mcc: tile[:, kc, mcc * 128:(mcc + 1) * 128]

            def xbar(tile):
                return lambda kc, mcc: tile[:, 4 * mcc + kc, :]

            def square(U_sl, S, name, copy_eng="dve", U_out=None):
                """A^{2m}: out[:, mcc, :] = sum_kc U_sl(kc,mcc)^T @ S[:,kc,:].
                With U_out, each mcc row is DMA-xbar-transposed into the
                next link's lhsT tile as soon as its copy lands."""
                Snew = cpool.tile([128, HC, H], bf16, tag=name)
                for mcc in range(HC):
                    ps = mmpool.tile([128, H], f32, tag="mm")
                    for kc in range(HC):
                        nc.tensor.matmul(
                            ps[:],
                            U_sl(kc, mcc),
                            S[:, kc, :],
                            start=(kc == 0),
                            stop=(kc == HC - 1),
                        )
                    if copy_eng == "act" or (copy_eng == "both" and mcc % 2):
                        nc.scalar.activation(Snew[:, mcc, :], ps[:], ACT_COPY)
                    else:
                        nc.vector.tensor_copy(Snew[:, mcc, :], ps[:])
                    if U_out is not None:
                        nc.sync.dma_start_transpose(
                            U_out[:, 4 * mcc:4 * mcc + 4, :],
                            Snew[:, mcc, :],
                        )
                return Snew

            def pe_transpose_mat(S, name):
                U = cpool.tile([128, HC, H], bf16, tag=name)
                for kc in range(HC):
                    transpose_quad(
                        U[:, kc, :],
                        [S[:, fc, kc * 128:(kc + 1) * 128] for fc in range(HC)],
                    )
                return nat(U)

            def c_round(c_prev, S_m, name):
                """c_{2m} = c_m + c_m @ A^m  (column form)."""
                psf = tlpool.tile([128, HC, 128], f32, tag="tl", name="csm")
                ps = psf[:, :, 0:1]
                for mcc in range(HC):
                    for kc in range(HC):
                        nc.tensor.matmul(
                            ps[:, mcc, :],
                            S_m[:, kc, mcc * 128:(mcc + 1) * 128],
                            c_prev[:, kc, :],
                            start=(kc == 0),
                            stop=(kc == HC - 1),
                        )
                c_new = cpool.tile([128, HC, 1], bf16, tag=name)
                nc.vector.tensor_add(c_new[:], ps[:], c_prev[:])
                return c_new

            def g_mats(lhs_sl, rhs, name, copy_eng="act"):
                """G[:, oc, :] = sum_kc lhs_sl(kc,oc)^T @ rhs(kc), 512 wide."""
                G = cpool.tile([128, ICH, H], bf16, tag=name)
                for oc in range(ICH):
                    ps = mmpool.tile([128, H], f32, tag="mm")
                    for kc in range(HC):
                        nc.tensor.matmul(
                            ps[:],
                            lhs_sl(kc, oc),
                            rhs(kc),
                            start=(kc == 0),
                            stop=(kc == HC - 1),
                        )
                    if copy_eng == "act":
                        nc.scalar.activation(G[:, oc, :], ps[:], ACT_COPY)
                    else:
                        nc.vector.tensor_copy(G[:, oc, :], ps[:])
                return G

            # ---- squaring chain / G matrices / bias vector -------------
            # The product chain A^2..A^128 is the latency backbone; all
            # projection and tree work is threaded between its links so
            # the PSUM->SBUF copies and DMA xbar transposes cost no PE
            # time.  Only the 7 Horner rounds remain as a serial tail.
            S2 = square(nat(w_bf), S1, "S2")        # U1 = Whh natural
            # Wq = Wxh^T as lhsT (fills the PE while S2's copies land)
            Wq = cpool.tile([128, ICH, H], bf16, tag="Wq")
            for ic in range(ICH):
                transpose_quad(
                    Wq[:, ic, :],
                    [wxh_bf[:, rc, ic * 128:(ic + 1) * 128] for rc in range(HC)],
                )
            # G1 = W A  (lhsT chunks = Wxh natural rows of wxh_bf)
            G1 = g_mats(
                lambda kc, oc: wxh_bf[:, kc, oc * 128:(oc + 1) * 128],
                lambda kc: S1[:, kc, :], "G1",
            )
            emit_g1t(G1)
            c2 = c_round(b_bf, S1, "c2")
            U2 = pe_transpose_mat(S2, "U2")
            G2 = g_mats(
                lambda kc, oc: wxh_bf[:, kc, oc * 128:(oc + 1) * 128],
                lambda kc: S2[:, kc, :], "G2", copy_eng="dve",
            )
            S4 = square(U2, S2, "S4")
            # G3 = G1 A^2 = W A^3
            G3 = g_mats(
                lambda kc, oc: G1T[:, 4 * oc + kc, :],
                lambda kc: S2[:, kc, :], "G3", copy_eng="dve",
            )
            U4 = pe_transpose_mat(S4, "U4")
            c4 = c_round(c2, S2, "c4")              # = b(I+A+A^2+A^3)

            PROJ_MATS = [Wq, G1, G2, G3]  # applied to t ≡ 3,2,1,0 (mod 4)
            # ---- main pipeline -----------------------------------------
            # Per x-group (512 timesteps of one batch row): radix-4 fused
            # projection -> 128 quad-columns.  Per half (4 groups, one per
            # row): tree levels A^4..A^64 collapse 512 -> 16 columns
            # (4 blocks of 128 timesteps x 4 rows, b-major).  A Horner
            # chain joins the 8 blocks with A^128.
            vhalf = {}
            v5 = {}

            pj_ps = {}

            def emit_proj(g, mccs=(0, 1, 2, 3)):
                half, row = g % 2, g // 2
                if half not in vhalf:
                    vhalf[half] = vpool.tile(
                        [128, HC, 512], bf16, tag=f"v0h{half}",
                        name=f"v0h{half}",
                    )
                if g in pj_ps:
                    pp = pj_ps.pop(g)
                else:
                    pp = pjpool.tile([128, HC, 128], f32, tag="pj")
                    pj_ps[g] = pp
                for mcc in mccs:
                    first = True
                    for m in range(4):
                        for ic in range(ICH):
                            nc.tensor.matmul(
                                pp[:, mcc, :],
                                PROJ_MATS[m][:, ic, mcc * 128:(mcc + 1) * 128],
                                xT_all[:, g, ic::2, 3 - m::4],
                                start=first,
                                stop=(m == 3 and ic == ICH - 1),
                            )
                            first = False
                # epilogue adds the radix-4 bias b(I+A+A^2+A^3)
                for mcc in mccs:
                    nc.scalar.activation(
                        vhalf[half][:, mcc, row * 128:(row + 1) * 128],
                        pp[:, mcc, :],
                        ACT_IDENT,
                        bias=c4[:, mcc, :],
                    )

            def tree_level(src, n_in, S_m, name, sub=None, dst=None):
                """One binary level over b-major cols: src [128, HC, n_in]
                -> dst [128, HC, n_in//2].  sub=(lo,hi) restricts batch
                rows (to keep a 512-col level's PSUM in one bank)."""
                b_lo, b_hi = sub if sub else (0, BC)
                nb = b_hi - b_lo
                per = n_in // BC
                srcr = src[:].rearrange("p c (b j) -> p c b j", b=BC)
                ps = tlpool.tile([128, HC, 128], f32, tag="tl")
                w = nb * per // 2
                for mcc in range(HC):
                    for kc in range(HC):
                        nc.tensor.matmul(
                            ps[:, mcc, 0:w],
                            S_m[:, kc, mcc * 128:(mcc + 1) * 128],
                            srcr[:, kc, b_lo:b_hi, 0::2],
                            start=(kc == 0),
                            stop=(kc == HC - 1),
                        )
                if dst is None:
                    dst = vpool.tile(
                        [128, HC, n_in // 2], bf16, tag=name, name=name
                    )
                dstr = dst[:].rearrange("p c (b j) -> p c b j", b=BC)
                nc.vector.tensor_add(
                    dstr[:, :, b_lo:b_hi, :],
                    ps[:, :, 0:w].rearrange("p c (b j) -> p c b j", b=nb),
                    srcr[:, :, b_lo:b_hi, 1::2],
                )
                return dst

            hs = {}

            def emit_horner(r):
                """h_r = h_{r-1} @ A^128 + V_r (V_r = row-block r%4 of
                half r//4); round 7 writes the f32 output tile."""
                vsrc = v5[r // 4][:].rearrange("p c (b j) -> p c b j", b=BC)
                rhs = (
                    v5[0][:].rearrange("p c (b j) -> p c b j", b=BC)
                    if r == 1 else hs[r - 1][:]
                )
                psf = tlpool.tile(
                    [128, HC, 128], f32, tag="tl", name=f"hsm{r % 2}"
                )
                ps = psf[:, :, 0:4]
                for mcc in range(HC):
                    for kc in range(HC):
                        nc.tensor.matmul(
                            ps[:, mcc, :],
                            S128[:, kc, mcc * 128:(mcc + 1) * 128],
                            rhs[:, kc, :, 0] if r == 1 else rhs[:, kc, :],
                            start=(kc == 0),
                            stop=(kc == HC - 1),
                        )
                if r < T // 128 - 1:
                    h = vpool.tile(
                        [128, HC, BC], bf16, tag="h", bufs=3, name="h"
                    )
                    nc.vector.tensor_add(h[:], ps[:], vsrc[:, :, :, r % 4])
                    hs[r] = h
                else:
                    fout = cpool.tile([128, HC, BC], f32, tag="fout")
                    nc.vector.tensor_add(fout[:], ps[:], vsrc[:, :, :, r % 4])
                    hs[r] = fout

            # thread everything through the product chain: ~one
            # projection pair fills each U transpose window; the tree
            # levels run as two alternating half-ladders so their
            # PSUM round trips hide behind each other
            U8t = cpool.tile([128, 16, 128], bf16, tag="U8")
            U16t = cpool.tile([128, 16, 128], bf16, tag="U16")
            U32t = cpool.tile([128, 16, 128], bf16, tag="U32")
            U64t = cpool.tile([128, 16, 128], bf16, tag="U64")
            emit_proj(0)
            S8 = square(U4, S4, "S8", U_out=U8t)
            U8 = xbar(U8t)
            # half-1 x transposes ride SP here: after U8's per-mcc
            # transposes (which gate S16) but well before proj(1) needs
            # them
            for g in (1, 3, 5, 7):
                emit_xpose(g)
            emit_proj(2)
            emit_proj(4)
            S16 = square(U8, S8, "S16", copy_eng="both", U_out=U16t)
            U16 = xbar(U16t)
            emit_proj(6)
            emit_proj(1)
            S32 = square(U16, S16, "S32", copy_eng="both", U_out=U32t)
            U32 = xbar(U32t)
            emit_proj(3)
            emit_proj(5)
            S64 = square(U32, S32, "S64", copy_eng="both", U_out=U64t)
            U64 = xbar(U64t)
            emit_proj(7)
            v0h0, v0h1 = vhalf[0], vhalf[1]
            v1h0 = tree_level(v0h0, 512, S4, "v1h0", sub=(0, 2))
            tree_level(v0h0, 512, S4, "v1h0", sub=(2, 4), dst=v1h0)
            S128 = square(U64, S64, "S128", copy_eng="both")
            v1h1 = tree_level(v0h1, 512, S4, "v1h1", sub=(0, 2))
            tree_level(v0h1, 512, S4, "v1h1", sub=(2, 4), dst=v1h1)
            v2h0 = tree_level(v1h0, 256, S8, "v2h0")
            v2h1 = tree_level(v1h1, 256, S8, "v2h1")
            v3h0 = tree_level(v2h0, 128, S16, "v3h0")
            v3h1 = tree_level(v2h1, 128, S16, "v3h1")
            v4h0 = tree_level(v3h0, 64, S32, "v4h0")
            v4h1 = tree_level(v3h1, 64, S32, "v4h1")
            v5[0] = tree_level(v4h0, 32, S64, "v5h0")
            emit_horner(1)
            v5[1] = tree_level(v4h1, 32, S64, "v5h1")
            emit_horner(2)
            for r in range(3, 8):
                emit_horner(r)

            nc.sync.dma_start(
                out_d.rearrange("p (c b) -> p c b", b=BC), hs[7][:]
            )

    nc.compile()
    return nc


def _get_nc():
    if "nc" not in _cache:
        _cache["nc"] = _build()
    return _cache["nc"]


def _in_maps(inputs):
    x = np.ascontiguousarray(np.asarray(inputs["x"], dtype=np.float32))
    wxh = np.ascontiguousarray(np.asarray(inputs["Wxh"], dtype=np.float32))
    bxh = np.ascontiguousarray(np.asarray(inputs["bxh"], dtype=np.float32))
    whh = np.ascontiguousarray(np.asarray(inputs["Whh"], dtype=np.float32))
    return [
        dict(
            x=np.ascontiguousarray(
                x[c * BC:(c + 1) * BC].reshape(COLS, IN)
            ),
            Wxh=wxh,
            bxh=bxh,
            Whh=whh,
        )
        for c in range(NCORES)
    ]


def kernel(**inputs) -> np.ndarray:
    from concourse.bass_utils import run_bass_kernel_spmd

    res = run_bass_kernel_spmd(
        _get_nc(), _in_maps(inputs), list(range(NCORES))
    ).results
    return _assemble(res)


def _assemble(results) -> np.ndarray:
    outs = []
    for c in range(NCORES):
        o = np.asarray(results[c]["h_out"])      # [128, HC*BC] on-chip layout
        o = o.reshape(128, HC, BC).transpose(2, 1, 0).reshape(BC, H)
        outs.append(o)
    return np.concatenate(outs, axis=0).astype(np.float32)
